# revision 7
# baseline (speedup 1.0000x reference)
"""Contrastive loss (cosine similarity) Trainium2 Bass kernel.

Shapes (hardcoded): anchor [1024, 4096] f32, positive [1024, 8, 4096] f32,
negative [1024, 64, 4096] f32. Output: scalar f32 loss.

Strategy: pure data-parallel over the batch dim across 8 NeuronCores
(128 rows each). Per core, stream the 72 candidate vectors (8 pos + 64 neg)
as [128, CH, 4096] tiles; for each sub-tile
  - DVE scalar_tensor_tensor: prod = v*a, dot = sum_free(prod)   (1 pass)
  - ACT activation(Square, accum_out): normsq = sum_free(v^2)    (1 pass)
Both engines overlap with the HBM DMA stream (~146 MB/core), which is the
roofline. A tiny epilogue computes per-row
  loss_b = logsumexp_j(sim[b, j]) - mean_p(sim[b, p])
and the host averages the 1024 per-row losses.

CH=2 makes each dma_start 4 MB with 32 KB contiguous per partition line
(half the descriptor count of CH=1), which measured better against the
straggling SDMA engine 15. Junk elementwise outputs (prod/sq) are single
shared tiles - WAW on one engine is program order, costs nothing, and the
freed SBUF goes to a deeper v-pool (4 x 4 MB in flight).
"""

import sys

if "/opt/trn_rl_repo" not in sys.path:
    sys.path.insert(0, "/opt/trn_rl_repo")

import numpy as np

import concourse.bass as bass
import concourse.mybir as mybir
import concourse.tile as tile
from concourse.bass_utils import run_bass_kernel_spmd

B, P, N, D = 1024, 8, 64, 4096
NCORES = 8
BS = B // NCORES  # 128 batch rows per core == SBUF partition count
J = P + N  # 72 candidates per row
TEMP = 0.1
CH = 2  # candidate vectors per DMA transfer for negatives (4 MB per dma_start)
VBUFS = 5

F32 = mybir.dt.float32
BF16 = mybir.dt.bfloat16
ALU = mybir.AluOpType
ACTF = mybir.ActivationFunctionType
AX = mybir.AxisListType


def build_bass():
    nc = bass.Bass()
    anchor = nc.dram_tensor("anchor", (BS, D), F32, kind="ExternalInput")
    positive = nc.dram_tensor("positive", (BS, P, D), F32, kind="ExternalInput")
    negative = nc.dram_tensor("negative", (BS, N, D), F32, kind="ExternalInput")
    losses = nc.dram_tensor("losses", (BS, 1), F32, kind="ExternalOutput")

    with tile.TileContext(nc) as tc:
        with (
            tc.tile_pool(name="vload", bufs=VBUFS) as vpool,
            tc.tile_pool(name="small", bufs=1) as small,
        ):
            # anchor rides the ACT ring so the first candidate transfer (on
            # the SP ring) lands concurrently -> DVE starts ~15us in, not 40
            a_tile = small.tile([BS, D], F32)
            nc.scalar.dma_start(out=a_tile, in_=anchor[:, :])

            dots = small.tile([BS, J], F32)
            # col J holds the anchor's squared norm so one sqrt covers all 73
            normsq = small.tile([BS, J + 1], F32)

            # single shared junk outputs; WAW per engine == program order.
            # bf16 halves their SBUF so the v-pool gets a 5th buffer.
            prod = small.tile([BS, D], BF16, tag="prod")
            sq = small.tile([BS, D], BF16, tag="sqd")

            nc.scalar.activation(
                out=sq, in_=a_tile, func=ACTF.Square, accum_out=normsq[:, J : J + 1]
            )

            # (tensor, start index, column base, vectors in this transfer).
            # Positives go as 8 small 2MB transfers to prime the pipeline
            # quickly; negatives stream as 32x 4MB.
            chunks = [(positive, p0, p0, 1) for p0 in range(P)]
            chunks += [(negative, n0, P + n0, CH) for n0 in range(0, N, CH)]

            for ci, (tens, i0, jbase, ch) in enumerate(chunks):
                v = vpool.tile([BS, ch, D], F32, tag="v")
                # all candidate loads issue from the SP sequencer: it runs no
                # compute, so dma_start dispatch (and its slot-wait) never
                # queues behind a square the way it does on the ACT ring
                nc.sync.dma_start(out=v, in_=tens[:, i0 : i0 + ch, :])
                for k in range(ch):
                    j = jbase + k
                    nc.vector.scalar_tensor_tensor(
                        out=prod,
                        in0=v[:, k, :],
                        scalar=1.0,
                        in1=a_tile,
                        op0=ALU.bypass,
                        op1=ALU.mult,
                        accum_out=dots[:, j : j + 1],
                    )
                    nc.scalar.activation(
                        out=sq,
                        in_=v[:, k, :],
                        func=ACTF.Square,
                        accum_out=normsq[:, j : j + 1],
                    )

            # Epilogue: sims = dots / (norm_j * norm_a * TEMP)
            norms = small.tile([BS, J + 1], F32)
            nc.scalar.activation(out=norms, in_=normsq, func=ACTF.Sqrt)
            rn = small.tile([BS, J + 1], F32)
            nc.vector.reciprocal(rn, norms)
            scaled = small.tile([BS, J], F32)
            nc.vector.tensor_scalar(
                out=scaled,
                in0=dots,
                scalar1=rn[:, J : J + 1],
                scalar2=1.0 / TEMP,
                op0=ALU.mult,
                op1=ALU.mult,
            )
            sims = small.tile([BS, J], F32)
            nc.vector.tensor_mul(sims, scaled, rn[:, 0:J])

            negm = small.tile([BS, 1], F32)
            nc.vector.tensor_reduce(
                out=negm, in_=sims, axis=AX.X, op=ALU.max, negate=True
            )
            exps = small.tile([BS, J], F32)
            sumexp = small.tile([BS, 1], F32)
            nc.scalar.activation(
                out=exps,
                in_=sims,
                func=ACTF.Exp,
                bias=negm,
                scale=1.0,
                accum_out=sumexp,
            )
            lnsum = small.tile([BS, 1], F32)
            nc.scalar.activation(out=lnsum, in_=sumexp, func=ACTF.Ln)

            pos_sum = small.tile([BS, 1], F32)
            nc.vector.tensor_reduce(out=pos_sum, in_=sims[:, 0:P], axis=AX.X, op=ALU.add)
            t2 = small.tile([BS, 1], F32)
            nc.vector.scalar_tensor_tensor(
                out=t2,
                in0=pos_sum,
                scalar=-1.0 / P,
                in1=lnsum,
                op0=ALU.mult,
                op1=ALU.add,
            )
            loss = small.tile([BS, 1], F32)
            nc.vector.tensor_sub(loss, t2, negm)
            nc.sync.dma_start(out=losses[:, :], in_=loss)

    return nc


def _split_waits_json(bir_bytes):
    """Rewrite BIR so no instruction carries more than one sync wait.

    The walrus build in this environment has a single sync-wait slot per ISA
    instruction ("Too many sync wait commands" otherwise). Tile emits 2-4
    waits on some instructions; hoist all but the last onto pure-wait
    EventSemaphore carrier instructions on the same engine, which preserves
    semantics (sequential waits on one engine == AND of conditions).
    """
    import json as _json

    bir = _json.loads(bir_bytes)
    ctr = 0
    for fn in bir["functions"]:
        for blk in fn["blocks"]:
            out = []
            for inst in blk["instructions"]:
                si = inst.get("sync_info")
                waits = (si or {}).get("on_wait") or []
                if len(waits) > 1:
                    for w in waits[:-1]:
                        ctr += 1
                        out.append(
                            {
                                "name": f"ws-{ctr}",
                                "opcode": "EventSemaphore",
                                "engine": inst["engine"],
                                "ins": [],
                                "outs": [],
                                "sync_info": {"on_update": [], "on_wait": [w]},
                            }
                        )
                    si["on_wait"] = waits[-1:]
                out.append(inst)
            blk["instructions"] = out
    return _json.dumps(bir).encode()


_NC_CACHE = None


def _get_nc():
    global _NC_CACHE
    if _NC_CACHE is None:
        nc = build_bass()
        orig = nc.to_json_bytes
        nc.to_json_bytes = lambda: _split_waits_json(orig())
        _NC_CACHE = nc
    return _NC_CACHE


def run(anchor, positive, negative, trace=False, trace_cores=None):
    """Run on 8 cores; returns (loss ndarray, BassKernelResults)."""
    anchor = np.ascontiguousarray(anchor, dtype=np.float32)
    positive = np.ascontiguousarray(positive, dtype=np.float32)
    negative = np.ascontiguousarray(negative, dtype=np.float32)
    in_maps = []
    for c in range(NCORES):
        sl = slice(c * BS, (c + 1) * BS)
        in_maps.append(
            {
                "anchor": np.ascontiguousarray(anchor[sl]),
                "positive": np.ascontiguousarray(positive[sl]),
                "negative": np.ascontiguousarray(negative[sl]),
            }
        )
    res = run_bass_kernel_spmd(
        _get_nc(),
        in_maps,
        core_ids=list(range(NCORES)),
        trace=trace,
        trace_cores=trace_cores,
    )
    losses = np.concatenate([r["losses"][:, 0] for r in res.results])
    out = np.asarray(losses.astype(np.float64).mean(), dtype=np.float32)
    return out, res


def kernel(anchor, positive, negative):
    out, _ = run(anchor, positive, negative)
    return out


# revision 14
# speedup vs baseline: 1.0944x; 1.0944x over previous
"""Contrastive loss (cosine similarity) Trainium2 Bass kernel.

Shapes (hardcoded): anchor [1024, 4096] f32, positive [1024, 8, 4096] f32,
negative [1024, 64, 4096] f32. Output: scalar f32 loss.

Strategy: pure data-parallel over the batch dim across 8 NeuronCores
(128 rows each). Per core, stream the 72 candidate vectors (8 pos + 64 neg)
as [128, CH, 4096] tiles; for each sub-tile
  - DVE scalar_tensor_tensor: prod = v*a, dot = sum_free(prod)   (1 pass)
  - ACT activation(Square, accum_out): normsq = sum_free(v^2)    (1 pass)
Both engines overlap with the HBM DMA stream (~146 MB/core), which is the
roofline. A tiny epilogue computes per-row
  loss_b = logsumexp_j(sim[b, j]) - mean_p(sim[b, p])
and the host averages the 1024 per-row losses.

CH=2 makes each dma_start 4 MB with 32 KB contiguous per partition line
(half the descriptor count of CH=1), which measured better against the
straggling SDMA engine 15. Junk elementwise outputs (prod/sq) are single
shared tiles - WAW on one engine is program order, costs nothing, and the
freed SBUF goes to a deeper v-pool (4 x 4 MB in flight).
"""

import sys

if "/opt/trn_rl_repo" not in sys.path:
    sys.path.insert(0, "/opt/trn_rl_repo")

import numpy as np

import concourse.bass as bass
import concourse.mybir as mybir
import concourse.tile as tile
from concourse.bass_utils import run_bass_kernel_spmd

B, P, N, D = 1024, 8, 64, 4096
NCORES = 8
BS = B // NCORES  # 128 batch rows per core == SBUF partition count
J = P + N  # 72 candidates per row
TEMP = 0.1
CH = 2  # candidate vectors per DMA transfer for negatives (4 MB per dma_start)
VBUFS = 5

F32 = mybir.dt.float32
BF16 = mybir.dt.bfloat16
ALU = mybir.AluOpType
ACTF = mybir.ActivationFunctionType
AX = mybir.AxisListType


def build_bass():
    nc = bass.Bass()
    anchor = nc.dram_tensor("anchor", (BS, D), F32, kind="ExternalInput")
    positive = nc.dram_tensor("positive", (BS, P, D), F32, kind="ExternalInput")
    negative = nc.dram_tensor("negative", (BS, N, D), F32, kind="ExternalInput")
    losses = nc.dram_tensor("losses", (4, 32), F32, kind="ExternalOutput")

    with tile.TileContext(nc) as tc:
        with (
            tc.tile_pool(name="vload", bufs=VBUFS) as vpool,
            tc.tile_pool(name="small", bufs=1) as small,
        ):
            # anchor rides the ACT ring so the first candidate transfer (on
            # the SP ring) lands concurrently -> DVE starts ~15us in, not 40
            a_tile = small.tile([BS, D], F32)
            nc.scalar.dma_start(out=a_tile, in_=anchor[:, :])

            dots = small.tile([BS, J], F32)
            # col J holds the anchor's squared norm so one sqrt covers all 73
            normsq = small.tile([BS, J + 1], F32)

            # single shared junk outputs; WAW per engine == program order.
            # bf16 halves their SBUF so the v-pool gets a 5th buffer.
            prod = small.tile([BS, D], BF16, tag="prod")
            sq = small.tile([BS, D], BF16, tag="sqd")

            nc.scalar.activation(
                out=sq, in_=a_tile, func=ACTF.Square, accum_out=normsq[:, J : J + 1]
            )

            # (tensor, start index, column base, vectors in this transfer).
            # Positives go as 8 small 2MB transfers to prime the pipeline
            # quickly; negatives stream as 32x 4MB.
            chunks = [(positive, p0, p0, 1) for p0 in range(P)]
            chunks += [(negative, n0, P + n0, CH) for n0 in range(0, N, CH)]

            # Software-pipelined trace order: each chunk's dma_start is
            # emitted LEAD chunks ahead of its compute so the ACT ring's
            # dma dispatches are never queued behind the square backlog
            # (the in-order ACT sequencer otherwise delays the final
            # transfers by ~20us of exposed ring-transit latency).
            LEAD = 3
            vtiles = [None] * len(chunks)

            def emit_compute(ci):
                tens, i0, jbase, ch = chunks[ci]
                v = vtiles[ci]
                for k in range(ch):
                    j = jbase + k
                    nc.vector.scalar_tensor_tensor(
                        out=prod,
                        in0=v[:, k, :],
                        scalar=1.0,
                        in1=a_tile,
                        op0=ALU.bypass,
                        op1=ALU.mult,
                        accum_out=dots[:, j : j + 1],
                    )
                    nc.scalar.activation(
                        out=sq,
                        in_=v[:, k, :],
                        func=ACTF.Square,
                        accum_out=normsq[:, j : j + 1],
                    )

            for ci, (tens, i0, jbase, ch) in enumerate(chunks):
                v = vpool.tile([BS, ch, D], F32, tag="v")
                vtiles[ci] = v
                # alternate between the two HWDGE rings (SP and ACT) to keep
                # more DMA requests outstanding toward the HBM controller
                dma_eng = nc.sync if ci % 2 == 0 else nc.scalar
                dma_eng.dma_start(out=v, in_=tens[:, i0 : i0 + ch, :])
                if ci >= LEAD:
                    emit_compute(ci - LEAD)
            for ci in range(len(chunks) - LEAD, len(chunks)):
                emit_compute(ci)

            # Epilogue: sims = dots / (norm_j * norm_a * TEMP)
            norms = small.tile([BS, J + 1], F32)
            nc.scalar.activation(out=norms, in_=normsq, func=ACTF.Sqrt)
            rn = small.tile([BS, J + 1], F32)
            nc.vector.reciprocal(rn, norms)
            scaled = small.tile([BS, J], F32)
            nc.vector.tensor_scalar(
                out=scaled,
                in0=dots,
                scalar1=rn[:, J : J + 1],
                scalar2=1.0 / TEMP,
                op0=ALU.mult,
                op1=ALU.mult,
            )
            sims = small.tile([BS, J], F32)
            nc.vector.tensor_mul(sims, scaled, rn[:, 0:J])

            negm = small.tile([BS, 1], F32)
            nc.vector.tensor_reduce(
                out=negm, in_=sims, axis=AX.X, op=ALU.max, negate=True
            )
            exps = small.tile([BS, J], F32)
            sumexp = small.tile([BS, 1], F32)
            nc.scalar.activation(
                out=exps,
                in_=sims,
                func=ACTF.Exp,
                bias=negm,
                scale=1.0,
                accum_out=sumexp,
            )
            lnsum = small.tile([BS, 1], F32)
            nc.scalar.activation(out=lnsum, in_=sumexp, func=ACTF.Ln)

            pos_sum = small.tile([BS, 1], F32)
            nc.vector.tensor_reduce(out=pos_sum, in_=sims[:, 0:P], axis=AX.X, op=ALU.add)
            t2 = small.tile([BS, 1], F32)
            nc.vector.scalar_tensor_tensor(
                out=t2,
                in0=pos_sum,
                scalar=-1.0 / P,
                in1=lnsum,
                op0=ALU.mult,
                op1=ALU.add,
            )
            # land the per-row loss in col 0 of a [128,32] tile, transpose
            # (DVE 32x32 blocks) so row 0 holds all 128 losses, and write
            # them out as ONE 512B descriptor instead of 128x 4B
            # read-modify-write descriptors (saves ~5us of completion wait)
            # block-transpose puts loss rows 32i..32i+31 into partition 32i,
            # cols 0..31 -> write out as 4x 128B descriptors instead of
            # 128x 4B read-modify-write descriptors (saves ~5us completion)
            loss_pad = small.tile([BS, 32], F32)
            nc.vector.tensor_sub(loss_pad[:, 0:1], t2, negm)
            loss_t = small.tile([BS, 32], F32)
            nc.vector.transpose(loss_t, loss_pad)
            nc.sync.dma_start(out=losses[:, :], in_=loss_t[0:128:32, :])

    return nc


def _split_waits_json(bir_bytes):
    """Rewrite BIR so no instruction carries more than one sync wait.

    The walrus build in this environment has a single sync-wait slot per ISA
    instruction ("Too many sync wait commands" otherwise). Tile emits 2-4
    waits on some instructions; hoist all but the last onto pure-wait
    EventSemaphore carrier instructions on the same engine, which preserves
    semantics (sequential waits on one engine == AND of conditions).
    """
    import json as _json

    bir = _json.loads(bir_bytes)
    ctr = 0
    for fn in bir["functions"]:
        for blk in fn["blocks"]:
            out = []
            for inst in blk["instructions"]:
                si = inst.get("sync_info")
                waits = (si or {}).get("on_wait") or []
                if len(waits) > 1:
                    for w in waits[:-1]:
                        ctr += 1
                        out.append(
                            {
                                "name": f"ws-{ctr}",
                                "opcode": "EventSemaphore",
                                "engine": inst["engine"],
                                "ins": [],
                                "outs": [],
                                "sync_info": {"on_update": [], "on_wait": [w]},
                            }
                        )
                    si["on_wait"] = waits[-1:]
                out.append(inst)
            blk["instructions"] = out
    return _json.dumps(bir).encode()


_NC_CACHE = None


def _get_nc():
    global _NC_CACHE
    if _NC_CACHE is None:
        nc = build_bass()
        orig = nc.to_json_bytes
        nc.to_json_bytes = lambda: _split_waits_json(orig())
        _NC_CACHE = nc
    return _NC_CACHE


def run(anchor, positive, negative, trace=False, trace_cores=None):
    """Run on 8 cores; returns (loss ndarray, BassKernelResults)."""
    anchor = np.ascontiguousarray(anchor, dtype=np.float32)
    positive = np.ascontiguousarray(positive, dtype=np.float32)
    negative = np.ascontiguousarray(negative, dtype=np.float32)
    in_maps = []
    for c in range(NCORES):
        sl = slice(c * BS, (c + 1) * BS)
        in_maps.append(
            {
                "anchor": np.ascontiguousarray(anchor[sl]),
                "positive": np.ascontiguousarray(positive[sl]),
                "negative": np.ascontiguousarray(negative[sl]),
            }
        )
    res = run_bass_kernel_spmd(
        _get_nc(),
        in_maps,
        core_ids=list(range(NCORES)),
        trace=trace,
        trace_cores=trace_cores,
    )
    losses = np.concatenate([r["losses"].reshape(BS) for r in res.results])
    out = np.asarray(losses.astype(np.float64).mean(), dtype=np.float32)
    return out, res


def kernel(anchor, positive, negative):
    out, _ = run(anchor, positive, negative)
    return out
